# revision 1
# baseline (speedup 1.0000x reference)
"""EventRNN (sparse_attention) Trainium2 Bass kernel.

Full-input contract: kernel(**inputs) takes the complete arrays from
setup_inputs() and returns the full (h_new[None], c_new[None]) tuple.

Sharding: data-parallel over batch B=32 across 8 NeuronCores (4 batches
per core); all weights replicated. Host-side prep is layout-only
(transposes / slicing / bool->additive mask); all FLOPs run on device.

Engine/dtype choices: big tensors (features, features_proj, LSTM weights)
ship as bf16 (memory-bound problem; references are bf16-envelope); PSUM
accumulation and softmax stay fp32; small attention logits matmuls use
fp32r. The additive mask is folded into the logits PSUM via a K=1 matmul.
LSTM bias rides the fused gates matmul as a 17th ones-row k-chunk.

Device program per core (b_loc = 4):
  phase A: q = h @ w_h2a.T + b_h2a  and  beta = sigmoid(h @ w_sel.T + b_sel)
           as PE matvecs in [d, b] layout.
  phase B: for each (batch, half):
             H = relu(projT_tile + q)          ACT, per-partition bias
             logits = w_att.T @ H              PE, contract over D in PSUM
             softmax row with additive mask    DVE reduce + ACT exp(accum)
             alphaT via PE transpose
             ctx = alphaT.T @ feats_tiles      PE, contract over L
           fc = beta/sum-scaled (past_ctx + future_ctx)
  phase C: gates = [cap|fc|feat|h] @ [W_ih|W_hh].T + b   one PE matmul chain
           LSTM elementwise on ACT/DVE, DMA h_new/c_new out.
"""

import numpy as np

import concourse.bacc as bacc
import concourse.mybir as mybir
import concourse.tile as tile
import concourse.masks as masks
from concourse.bass_utils import run_bass_kernel_spmd

F32 = mybir.dt.float32
F32R = mybir.dt.float32r
BF16 = mybir.dt.bfloat16
AF = mybir.ActivationFunctionType
ALU = mybir.AluOpType

B, L, D, H = 32, 2048, 512, 512
N_CORES = 8
B_LOC = B // N_CORES          # 4 batches per core
FIDX = 1024                   # static feature_idx from setup_inputs()
HALF = L // 2                 # past/future split == 1024
P = 128
DC = D // P                   # 4 d-chunks
KC = (H + 2 * D + H) // P     # 16 k-chunks for the fused LSTM matmul
KC_G = KC + 1                 # +1 bias chunk (ones-row trick)
G4 = 4 * H                    # 2048 gate columns
LC = HALF // P                # 8 L-chunks of 128 per half
LS = HALF // 512              # 2 N-segments of 512 per half


def build_nc():
    nc = bacc.Bacc("TRN2", target_bir_lowering=False, debug=False,
                   num_devices=N_CORES)

    # ---- DRAM I/O ----
    projT = nc.dram_tensor("projT", [B_LOC, D, L], BF16, kind="ExternalInput").ap()
    feats = nc.dram_tensor("feats", [B_LOC, L, D], BF16, kind="ExternalInput").ap()
    WT = nc.dram_tensor("WT", [KC_G * P, G4], BF16, kind="ExternalInput").ap()
    w_h2aT = nc.dram_tensor("w_h2aT", [H, D], BF16, kind="ExternalInput").ap()
    w_pf = nc.dram_tensor("w_pf", [D, 2], F32R, kind="ExternalInput").ap()
    w_selT = nc.dram_tensor("w_selT", [H, 1], BF16, kind="ExternalInput").ap()
    b_h2a = nc.dram_tensor("b_h2a", [D, 1], F32, kind="ExternalInput").ap()
    b_sel = nc.dram_tensor("b_sel", [1, 1], F32, kind="ExternalInput").ap()
    maskadd = nc.dram_tensor("maskadd", [2 * B_LOC, HALF], BF16,
                             kind="ExternalInput").ap()
    capT = nc.dram_tensor("capT", [H, B_LOC], BF16, kind="ExternalInput").ap()
    featT = nc.dram_tensor("featT", [D, B_LOC], BF16, kind="ExternalInput").ap()
    hT = nc.dram_tensor("hT", [H, B_LOC], BF16, kind="ExternalInput").ap()
    c_last = nc.dram_tensor("c_last", [B_LOC, H], F32, kind="ExternalInput").ap()
    h_out = nc.dram_tensor("h_new", [B_LOC, H], F32, kind="ExternalOutput").ap()
    c_out = nc.dram_tensor("c_new", [B_LOC, H], F32, kind="ExternalOutput").ap()

    with tile.TileContext(nc) as tc:
        with tc.tile_pool(name="const", bufs=1) as const, \
             tc.tile_pool(name="wres", bufs=1) as wres:
            # ---- resident constants ----
            ident = const.tile([P, P], F32)
            masks.make_identity(nc, ident[:])
            ones_bf = const.tile([1, 1], BF16)
            nc.gpsimd.memset(ones_bf[:], 1.0)
            madd_sb = const.tile([1, 2 * B_LOC * HALF], BF16)
            nc.sync.dma_start(madd_sb[:], maskadd.rearrange("r l -> (r l)").unsqueeze(0))

            w_h2aT_sb = const.tile([P, H // P, D], BF16)
            nc.sync.dma_start(w_h2aT_sb[:], w_h2aT.rearrange("(c p) n -> p c n", p=P))
            w_pf_sb = const.tile([P, DC, 2], F32R)
            nc.sync.dma_start(w_pf_sb[:], w_pf.rearrange("(c p) n -> p c n", p=P))
            w_selT_sb = const.tile([P, H // P, 1], BF16)
            nc.sync.dma_start(w_selT_sb[:], w_selT.rearrange("(c p) n -> p c n", p=P))
            b_h2a_sb = const.tile([P, DC], F32)
            nc.sync.dma_start(b_h2a_sb[:], b_h2a.rearrange("(c p) n -> p (c n)", p=P))
            b_sel_sb = const.tile([1, 1], F32)
            nc.sync.dma_start(b_sel_sb[:], b_sel[:])
            # xhT = [caption | fc | feature | h_last] transposed: [128, 16, 4]
            xhT = const.tile([P, KC_G, B_LOC], BF16)
            nc.gpsimd.memset(xhT[:, 16, :], 0.0)
            nc.gpsimd.memset(xhT[0:1, 16, :], 1.0)
            nc.sync.dma_start(xhT[:, 0:4, :], capT.rearrange("(c p) n -> p c n", p=P))
            nc.sync.dma_start(xhT[:, 8:12, :], featT.rearrange("(c p) n -> p c n", p=P))
            nc.sync.dma_start(xhT[:, 12:16, :], hT.rearrange("(c p) n -> p c n", p=P))

            # resident LSTM weights [128, 16, 2048] (128 KB / partition)
            WT_sb = wres.tile([P, KC_G, G4], BF16)

            # softmax / context workspace (all partition-base-0;
            # per-(b,half) scalars live in the FREE dim, r = h*4+b)
            negm = const.tile([1, 2 * B_LOC], F32)
            sums = const.tile([1, 2 * B_LOC], F32)
            recips = const.tile([1, 2 * B_LOC], F32)
            svals = const.tile([1, 2 * B_LOC], F32)
            alphaT = const.tile([P, 2, LC, B_LOC], BF16)
            qb = const.tile([P, DC * B_LOC], F32)
            beta_sb = const.tile([1, B_LOC], F32)

            # ================= phase A: q and beta matvecs =================
            with tc.tile_pool(name="psA", bufs=1, space="PSUM") as psA:
                q_ps = psA.tile([P, DC * B_LOC], F32)
                beta_ps = psA.tile([1, B_LOC], F32)
                for dc in range(DC):
                    for kc in range(H // P):
                        nc.tensor.matmul(
                            q_ps[:, dc * B_LOC:(dc + 1) * B_LOC],
                            w_h2aT_sb[:, kc, dc * P:(dc + 1) * P],
                            xhT[:, 12 + kc, :],
                            start=(kc == 0), stop=(kc == H // P - 1))
                    nc.scalar.activation(
                        qb[:, dc * B_LOC:(dc + 1) * B_LOC],
                        q_ps[:, dc * B_LOC:(dc + 1) * B_LOC],
                        AF.Identity, bias=b_h2a_sb[:, dc:dc + 1])
                for kc in range(H // P):
                    nc.tensor.matmul(beta_ps[:], w_selT_sb[:, kc, :],
                                     xhT[:, 12 + kc, :],
                                     start=(kc == 0), stop=(kc == H // P - 1))
                nc.scalar.activation(beta_sb[:], beta_ps[:], AF.Sigmoid,
                                     bias=b_sel_sb[0:1, 0:1])

            # ================= phase B: attention =================
            with tc.tile_pool(name="proj", bufs=4) as projp, \
                 tc.tile_pool(name="hatt", bufs=6) as hattp, \
                 tc.tile_pool(name="fpool", bufs=4) as fpool, \
                 tc.tile_pool(name="rowp", bufs=3) as rowp, \
                 tc.tile_pool(name="fcpool", bufs=2) as fcpool, \
                 tc.tile_pool(name="pslog", bufs=1, space="PSUM") as pslog, \
                 tc.tile_pool(name="pst", bufs=1, space="PSUM") as pst, \
                 tc.tile_pool(name="psctx", bufs=1, space="PSUM") as psctx, \
                 tc.tile_pool(name="psg", bufs=2, space="PSUM") as psg:

                # fused LSTM gates accumulate during attention; each
                # k-chunk's matmuls are emitted right after its WT DMA
                g_ps1 = psg.tile([B_LOC, 2 * H], F32, tag="g")
                g_ps2 = psg.tile([B_LOC, 2 * H], F32, tag="g")

                fcA = {}
                for b in range(B_LOC):
                    for h in range(2):
                        r = h * B_LOC + b
                        # interleave resident-weight loads with the big loop
                        lg_ps = pslog.tile([1, HALF], F32)
                        # preload additive mask into the logits psum via a
                        # K=1 matmul; logits then accumulate on top
                        for ls in range(LS):
                            nc.tensor.matmul(
                                lg_ps[:, ls * 512:(ls + 1) * 512],
                                ones_bf[0:1, 0:1],
                                madd_sb[0:1, r * HALF + ls * 512:
                                        r * HALF + (ls + 1) * 512],
                                start=True, stop=False)
                        hatts = {}
                        for dp in range(DC // 2):
                            projt = projp.tile([P, 2, HALF], BF16)
                            nc.sync.dma_start(
                                projt[:],
                                projT[b, dp * 2 * P:(dp + 1) * 2 * P,
                                      h * HALF:(h + 1) * HALF]
                                .rearrange("(j p) l -> p j l", p=P))
                            for jj in range(2):
                                dc = dp * 2 + jj
                                hatt = hattp.tile([P, HALF], F32R)
                                nc.scalar.activation(
                                    hatt[:], projt[:, jj, :], AF.Relu,
                                    bias=qb[:, dc * B_LOC + b:
                                            dc * B_LOC + b + 1])
                                hatts[dc] = hatt
                        for ls in range(LS):
                            for dc in range(DC):
                                nc.tensor.matmul(
                                    lg_ps[:, ls * 512:(ls + 1) * 512],
                                    w_pf_sb[:, dc, h:h + 1],
                                    hatts[dc][:, ls * 512:(ls + 1) * 512],
                                    start=False, stop=(dc == DC - 1))
                        # row softmax straight from psum
                        nc.vector.tensor_reduce(
                            negm[0:1, r:r + 1], lg_ps[0:1, :],
                            axis=mybir.AxisListType.X, op=ALU.max, negate=True)
                        alpha_r = rowp.tile([1, HALF], F32, tag="alpha")
                        nc.scalar.activation(
                            alpha_r[:], lg_ps[0:1, :], AF.Exp,
                            bias=negm[0:1, r:r + 1],
                            accum_out=sums[0:1, r:r + 1])
                        nc.vector.reciprocal(recips[0:1, r:r + 1],
                                             sums[0:1, r:r + 1])
                        nc.vector.tensor_tensor(svals[0:1, r:r + 1],
                                                recips[0:1, r:r + 1],
                                                beta_sb[0:1, b:b + 1],
                                                op=ALU.mult)
                        # transpose alpha row into [128, lc] columns
                        for lc in range(LC):
                            tr_ps = pst.tile([P, 1], F32)
                            nc.tensor.transpose(
                                tr_ps[:, 0:1],
                                alpha_r[0:1, lc * P:(lc + 1) * P],
                                ident[0:1, 0:1])
                            nc.vector.tensor_copy(alphaT[:, h, lc, b:b + 1],
                                                  tr_ps[:])
                        # context matvec, contract over L
                        ctx_ps = psctx.tile([1, D], F32)
                        for lq in range(2):
                            featst = fpool.tile([P, 4, D], BF16)
                            nc.sync.dma_start(
                                featst[:],
                                feats[b, h * HALF + lq * 4 * P:
                                      h * HALF + (lq + 1) * 4 * P, :]
                                .rearrange("(j p) d -> p j d", p=P))
                            for jj in range(4):
                                lc = lq * 4 + jj
                                nc.tensor.matmul(
                                    ctx_ps[:], alphaT[:, h, lc, b:b + 1],
                                    featst[:, jj, :],
                                    start=(lc == 0), stop=(lc == LC - 1))
                        if h == 0:
                            # stash s_p * ctx_p, freeing the psum tile
                            fcA_b = fcpool.tile([1, D], F32, tag="fcA", bufs=4)
                            nc.vector.tensor_scalar_mul(
                                fcA_b[:], ctx_ps[0:1, :], svals[0:1, b:b + 1])
                            fcA[b] = fcA_b
                        else:
                            # fc_b = s_f * ctx_f + fcA_b, then -> xhT (transposed)
                            fc_b = fcpool.tile([1, D], F32, tag="fcB", bufs=2)
                            nc.vector.scalar_tensor_tensor(
                                fc_b[:], ctx_ps[0:1, :],
                                svals[0:1, B_LOC + b:B_LOC + b + 1], fcA[b][:],
                                op0=ALU.mult, op1=ALU.add)
                            for dc in range(DC):
                                tr_ps = pst.tile([P, 1], F32)
                                nc.tensor.transpose(
                                    tr_ps[:, 0:1],
                                    fc_b[0:1, dc * P:(dc + 1) * P],
                                    ident[0:1, 0:1])
                                nc.vector.tensor_copy(xhT[:, 4 + dc, b:b + 1],
                                                      tr_ps[:])
                        # weight loads + filler gates matmuls at low
                        # priority (end of each iteration body)
                        base = (b * 2 + h) * 2
                        nc.sync.dma_start(
                            WT_sb[:, base:base + 2, :],
                            WT[base * P:(base + 2) * P, :]
                            .rearrange("(j p) n -> p j n", p=P))
                        ws = [base, base + 1] + ([16] if base == 0 else [])
                        if base == 0:
                            nc.sync.dma_start(WT_sb[:, 16, :],
                                              WT[16 * P:17 * P, :])
                        for wkc in ws:
                            if wkc not in (4, 5, 6, 7):
                                for ns in range(2):
                                    nc.tensor.matmul(
                                        g_ps1[:, ns * 512:(ns + 1) * 512],
                                        xhT[:, wkc, :],
                                        WT_sb[:, wkc, ns * 512:(ns + 1) * 512],
                                        start=(wkc == 0), stop=False)
                                    nc.tensor.matmul(
                                        g_ps2[:, ns * 512:(ns + 1) * 512],
                                        xhT[:, wkc, :],
                                        WT_sb[:, wkc,
                                              (2 + ns) * 512:(3 + ns) * 512],
                                        start=(wkc == 0), stop=False)

            # ================= phase C: fc-dependent gates + LSTM ==========
                lstm = const  # reuse the const pool scope for LSTM tiles
                c_last_sb = lstm.tile([B_LOC, H], F32)
                nc.sync.dma_start(c_last_sb[:], c_last[:])

                for ki, kc in enumerate((4, 5, 6, 7)):
                    for ns in range(2):
                        nc.tensor.matmul(
                            g_ps1[:, ns * 512:(ns + 1) * 512],
                            xhT[:, kc, :],
                            WT_sb[:, kc, ns * 512:(ns + 1) * 512],
                            start=False, stop=(ki == 3))
                for ki, kc in enumerate((4, 5, 6, 7)):
                    for ns in range(2):
                        nc.tensor.matmul(
                            g_ps2[:, ns * 512:(ns + 1) * 512],
                            xhT[:, kc, :],
                            WT_sb[:, kc, (2 + ns) * 512:(3 + ns) * 512],
                            start=False, stop=(ki == 3))
                # gate rows reordered [i, f, o, g]; bias folded into matmul
                g_sb = lstm.tile([B_LOC, G4], F32)
                nc.scalar.activation(g_sb[:, 0:2 * H], g_ps1[:, 0:2 * H],
                                     AF.Sigmoid)
                # f * c_last can run while the second gates half accumulates
                c_new = lstm.tile([B_LOC, H], F32)
                nc.vector.tensor_tensor(c_new[:], g_sb[:, H:2 * H], c_last_sb[:],
                                        op=ALU.mult)

                # tanh(x) = 2*sigmoid(2x) - 1: stays on the sigmoid ACT
                # table (avoids two table loads in the latency-critical tail)
                nc.scalar.activation(g_sb[:, 3 * H:4 * H], g_ps2[:, H:2 * H],
                                     AF.Sigmoid, scale=2.0)
                nc.vector.tensor_scalar(g_sb[:, 3 * H:4 * H],
                                        g_sb[:, 3 * H:4 * H], 2.0, -1.0,
                                        op0=ALU.mult, op1=ALU.add)
                nc.scalar.activation(g_sb[:, 2 * H:3 * H], g_ps2[:, 0:H],
                                     AF.Sigmoid)

                t2 = lstm.tile([B_LOC, H], F32)
                h_new = lstm.tile([B_LOC, H], F32)
                nc.vector.tensor_tensor(t2[:], g_sb[:, 0:H], g_sb[:, 3 * H:4 * H],
                                        op=ALU.mult)
                nc.vector.tensor_tensor(c_new[:], c_new[:], t2[:], op=ALU.add)
                nc.scalar.activation(t2[:], c_new[:], AF.Sigmoid, scale=2.0)
                nc.vector.tensor_scalar(t2[:], t2[:], 2.0, -1.0,
                                        op0=ALU.mult, op1=ALU.add)
                nc.vector.tensor_tensor(h_new[:], g_sb[:, H * 2:H * 3], t2[:],
                                        op=ALU.mult)

                nc.sync.dma_start(c_out[:], c_new[:])
                nc.sync.dma_start(h_out[:], h_new[:])

    nc.compile()
    return nc


_NC_CACHE = None


def _get_nc():
    global _NC_CACHE
    if _NC_CACHE is None:
        _NC_CACHE = build_nc()
    return _NC_CACHE


def make_in_maps(features, features_proj, hidden_states, cell_states,
                 caption_hidden_states, w_h2a, b_h2a, w_patt, b_patt,
                 w_fatt, b_fatt, w_sel, b_sel, w_ih, w_hh, b_ih, b_hh,
                 mask, feature_idx):
    assert int(feature_idx) == FIDX
    import ml_dtypes
    f32 = np.float32
    bf16 = ml_dtypes.bfloat16
    features = np.asarray(features, f32)
    features_proj = np.asarray(features_proj, f32)
    h_last = np.asarray(hidden_states, f32)[-1]          # [B, H]
    c_last = np.asarray(cell_states, f32)[-1]            # [B, H]
    cap = np.asarray(caption_hidden_states, f32)         # [B, H]
    mask = np.asarray(mask)

    # shared (replicated) tensors — layout-only host prep
    Wfull = np.concatenate([np.asarray(w_ih, f32), np.asarray(w_hh, f32)], axis=1)
    gate_perm = np.r_[0:512, 512:1024, 1536:2048, 1024:1536]
    b_ihh = (np.asarray(b_ih, f32) + np.asarray(b_hh, f32))[gate_perm]
    WTf = np.zeros((KC_G * 128, 4 * H), f32)
    WTf[0:2048] = Wfull[gate_perm].T
    WTf[2048] = b_ihh
    WT = np.ascontiguousarray(WTf).astype(bf16)
    w_h2aT = np.ascontiguousarray(np.asarray(w_h2a, f32).T).astype(bf16)
    w_pf = np.ascontiguousarray(
        np.stack([np.asarray(w_patt, f32)[0], np.asarray(w_fatt, f32)[0]], axis=1))
    w_selT = np.ascontiguousarray(np.asarray(w_sel, f32).T).astype(bf16)
    b_h2a_c = np.ascontiguousarray(np.asarray(b_h2a, f32)[:, None])  # [D, 1]
    b_sel_c = np.asarray(b_sel, f32).reshape(1, 1)
    # additive mask, rows (half, b): 0 where visible, -1e30 where masked
    madd = np.where(mask, f32(0), f32(-1e30)).astype(bf16)           # [B, L]

    in_maps = []
    for c in range(N_CORES):
        sl = slice(c * B_LOC, (c + 1) * B_LOC)
        m = madd[sl].reshape(B_LOC, 2, HALF).transpose(1, 0, 2)      # [2, 4, HALF]
        in_maps.append({
            "projT": np.ascontiguousarray(features_proj[sl].transpose(0, 2, 1)).astype(bf16),
            "feats": np.ascontiguousarray(features[sl]).astype(bf16),
            "WT": WT,
            "w_h2aT": w_h2aT,
            "w_pf": w_pf,
            "w_selT": w_selT,
            "b_h2a": b_h2a_c,
            "b_sel": b_sel_c,
            "maskadd": np.ascontiguousarray(m.reshape(2 * B_LOC, HALF)),
            "capT": np.ascontiguousarray(cap[sl].T).astype(bf16),
            "featT": np.ascontiguousarray(features[sl, FIDX, :].T).astype(bf16),
            "hT": np.ascontiguousarray(h_last[sl].T).astype(bf16),
            "c_last": np.ascontiguousarray(c_last[sl]),
        })
    return in_maps


def run(trace=False, **inputs):
    nc = _get_nc()
    in_maps = make_in_maps(**inputs)
    res = run_bass_kernel_spmd(nc, in_maps, core_ids=list(range(N_CORES)),
                               trace=trace)
    h = np.concatenate([res.results[c]["h_new"] for c in range(N_CORES)], axis=0)
    c = np.concatenate([res.results[c]["c_new"] for c in range(N_CORES)], axis=0)
    return (h[None], c[None]), res


def kernel(**inputs):
    out, _ = run(trace=False, **inputs)
    return out

